# revision 34
# baseline (speedup 1.0000x reference)
"""GQA attention layer (B=2, S=2048, DM=2048, H=32, KV=8, HD=64) with RoPE,
causal mask and output projection, tensor-parallel over heads across 8
Trainium2 NeuronCores.

Sharding: core c owns query heads [4c, 4c+4) and KV head c.  Each core
computes its 4 heads' attention plus its slice of the output projection
(W_o rows [256c, 256c+256)); the host sums the 8 partial outputs and adds
b_o.

Layout strategy: everything on-chip lives transposed (feature dim on
partitions) so that Q/K projections, RoPE rotation (a PE matmul against a
+-1 permutation matrix), scores A^T = K^T.T @ Q^T, P^T = exp(A^T/8), and
O'^T = V'.T @ P^T chain together without any transposes except V (done
with 16 small PE transposes per batch).  The softmax denominator rides as
a ones-column appended to V.  Matmuls run in float32r (~1.6e-4 rel err,
4x the fp32 rate).
"""

import os
import numpy as np

import concourse.bass as bass
import concourse.tile as tile
from concourse import bacc, mybir
from concourse.bass_utils import run_bass_kernel_spmd

F32 = mybir.dt.float32
F32R = mybir.dt.float32r
EXP = mybir.ActivationFunctionType.Exp
MULT = mybir.AluOpType.add  # placeholder (unused)

B, S, DM = 2, 2048, 2048
H, KV, HD = 32, 8, 64
NCORES = 8
HPC = H // NCORES            # query heads per core
BS = B * S
KSUB = DM // 128             # 16 contraction subtiles
NCH = S // 512               # 4 seq chunks of 512 per batch
SK = S // 128                # 16 key subtiles per batch

_NC = None
LAST_EXEC_NS = None


def _build():
    nc = bacc.Bacc("TRN2", target_bir_lowering=False, debug=False)
    d = {}
    d["xt"] = nc.declare_dram_parameter("xt", [128, KSUB, BS], F32R, isOutput=False)
    d["wq"] = nc.declare_dram_parameter("wq", [128, KSUB, 256], F32R, isOutput=False)
    d["wkv"] = nc.declare_dram_parameter("wkv", [128, KSUB, 128], F32R, isOutput=False)
    d["wo"] = nc.declare_dram_parameter("wo", [128, 2, DM], F32R, isOutput=False)
    d["cos2t"] = nc.declare_dram_parameter("cos2t", [128, BS], F32, isOutput=False)
    d["sin2t"] = nc.declare_dram_parameter("sin2t", [128, BS], F32, isOutput=False)
    d["bv"] = nc.declare_dram_parameter("bv", [128, 1], F32, isOutput=False)
    d["rt"] = nc.declare_dram_parameter("rt", [128, 128], F32R, isOutput=False)
    d["idl"] = nc.declare_dram_parameter("idl", [128, 64], F32R, isOutput=False)
    d["m0"] = nc.declare_dram_parameter("m0", [128, 128], F32R, isOutput=False)
    d["on16"] = nc.declare_dram_parameter("on16", [128, SK], F32R, isOutput=False)
    out_d = nc.declare_dram_parameter("out", [BS, DM], F32, isOutput=True)

    import contextlib
    with tile.TileContext(nc) as tc:
        ctx = contextlib.ExitStack()
        cst = ctx.enter_context(tc.tile_pool(name="cst", bufs=1))
        act = ctx.enter_context(tc.tile_pool(name="act", bufs=1))
        qTp = ctx.enter_context(tc.tile_pool(name="qTp", bufs=1))
        big = ctx.enter_context(tc.tile_pool(name="big", bufs=2))
        ptp = ctx.enter_context(tc.tile_pool(name="ptp", bufs=11))
        sm1 = ctx.enter_context(tc.tile_pool(name="sm1", bufs=1))
        sm2 = ctx.enter_context(tc.tile_pool(name="sm2", bufs=2))
        ost_p = ctx.enter_context(tc.tile_pool(name="ostp", bufs=2))
        dp = ctx.enter_context(tc.tile_pool(name="dp", bufs=2, space="DRAM"))
        pmm = ctx.enter_context(tc.tile_pool(name="pmm", bufs=1, space="PSUM"))
        pa = ctx.enter_context(tc.tile_pool(name="pa", bufs=3, space="PSUM"))
        po = ctx.enter_context(tc.tile_pool(name="po", bufs=4, space="PSUM"))

        wq = cst.tile([128, KSUB, 256], F32R)
        wkv = cst.tile([128, KSUB, 128], F32R)
        wo = cst.tile([128, 2, DM], F32R)
        bv = cst.tile([128, 1], F32)
        rt = cst.tile([128, 128], F32R)
        idl = cst.tile([128, 64], F32R)
        m0 = cst.tile([128, 128], F32R)
        nc.sync.dma_start(out=wq[:, 0:8], in_=d["wq"][:, 0:8])
        nc.scalar.dma_start(out=wq[:, 8:16], in_=d["wq"][:, 8:16])
        nc.gpsimd.dma_start(out=wkv[:], in_=d["wkv"][:])
        for t, key in ((wo, "wo"), (bv, "bv"), (rt, "rt"), (idl, "idl"),
                       (m0, "m0")):
            nc.gpsimd.dma_start(out=t[:], in_=d[key][:])

        CW = 512  # QKV streaming chunk width

        def rope(dst, psrc, rows, cosc, sinc, rt_sl):
            """dst[rows, cols] = psrc*cos + rot(psrc)*sin, rot via PE."""
            sb = sm2.tile([128, CW], F32R, tag="qsb")
            nc.vector.tensor_copy(out=sb[0:rows, :], in_=psrc)
            prot = pa.tile([128, 512], F32, tag="a")
            nc.tensor.matmul(prot[0:rows, 0:CW], lhsT=rt_sl, rhs=sb[0:rows, :],
                             start=True, stop=True)
            nc.vector.tensor_mul(out=dst, in0=sb[0:rows, :], in1=cosc[0:rows, :])
            rots = sm1.tile([128, CW], F32, tag="rots")
            nc.vector.tensor_mul(out=rots[0:rows, :], in0=prot[0:rows, 0:CW],
                                 in1=sinc[0:rows, :])
            nc.vector.tensor_add(out=dst, in0=dst, in1=rots[0:rows, :])

        def emit_outproj(b, attnT, qt_i):
            for ss in range(qt_i * 4, qt_i * 4 + 4):
                for oc in range(DM // 512):
                    ps = pmm.tile([128, 512], F32, tag="mm")
                    for k2 in range(2):
                        nc.tensor.matmul(ps[:],
                                         lhsT=attnT[:, k2, ss * 128:(ss + 1) * 128],
                                         rhs=wo[:, k2, oc * 512:(oc + 1) * 512],
                                         start=(k2 == 0), stop=(k2 == 1))
                    ost = ost_p.tile([128, 512], F32, tag="ost")
                    nc.vector.tensor_copy(out=ost[:], in_=ps[:])
                    nc.scalar.dma_start(
                        out=out_d[b * S + ss * 128:b * S + (ss + 1) * 128,
                                  oc * 512:(oc + 1) * 512],
                        in_=ost[:])

        deferred = []
        for b in range(B):
            qT = qTp.tile([128, 2, S], F32R, tag="qT")
            kTd = act.tile([128, S], F32R, tag="kTd")
            vp = act.tile([128, SK, 65], F32R, tag="vp")
            nc.gpsimd.dma_start(out=vp[:, :, 64], in_=d["on16"][:])
            attnT = act.tile([128, 2, S], F32R, tag="attnT")

            # ---- QKV projections + RoPE + V transpose ----
            for sc in range(S // CW):
                col0 = b * S + sc * CW
                cs = slice(sc * CW, sc * CW + CW)
                xt = big.tile([128, KSUB, CW], F32R, tag="big")
                nc.sync.dma_start(out=xt[:, 0:8], in_=d["xt"][:, 0:8, col0:col0 + CW])
                nc.scalar.dma_start(out=xt[:, 8:16],
                                    in_=d["xt"][:, 8:16, col0:col0 + CW])
                cosc = sm1.tile([128, CW], F32, tag="cos")
                sinc = sm1.tile([128, CW], F32, tag="sin")
                nc.gpsimd.dma_start(out=cosc[:], in_=d["cos2t"][:, col0:col0 + CW])
                nc.gpsimd.dma_start(out=sinc[:], in_=d["sin2t"][:, col0:col0 + CW])
                ps = pmm.tile([128, CW], F32, tag="mm")
                for ks in range(KSUB):
                    nc.tensor.matmul(ps[:], lhsT=wkv[:, ks, :], rhs=xt[:, ks, :],
                                     start=(ks == 0), stop=(ks == KSUB - 1))
                rope(kTd[0:64, cs], ps[0:64, :], 64, cosc, sinc, rt[0:64, 0:64])
                nc.sync.dma_start(out=kTd[64:128, cs], in_=kTd[0:64, cs])
                vtt = sm1.tile([128, CW], F32R, tag="vtt")
                nc.vector.tensor_add(out=vtt[64:128, :], in0=ps[64:128, :],
                                     in1=bv[64:128, 0:1].to_broadcast((64, CW)))
                for t4 in range(CW // 128):
                    pvt = pa.tile([128, 64], F32R, tag="a")
                    nc.tensor.matmul(pvt[:], lhsT=vtt[64:128, t4 * 128:(t4 + 1) * 128],
                                     rhs=idl[64:128, :], is_transpose=True,
                                     start=True, stop=True)
                    nc.vector.tensor_copy(
                        out=vp[:, (sc * CW) // 128 + t4, 0:64], in_=pvt[:])
                for m in range(2):
                    ps = pmm.tile([128, CW], F32, tag="mm")
                    for ks in range(KSUB):
                        nc.tensor.matmul(ps[:], lhsT=wq[:, ks, m * 128:(m + 1) * 128],
                                         rhs=xt[:, ks, :],
                                         start=(ks == 0), stop=(ks == KSUB - 1))
                    rope(qT[:, m, cs], ps[:], 128, cosc, sinc, rt[:])

            # ---- attention: A/exp stream leads; AV matmuls trail by a
            # W-step window and spill across (qt,qsub) unit boundaries so the
            # PE always has ready (never just-in-time) work ----
            def norm_out(pov, qrow, qsub, qs0):
                l65 = sm2.tile([65, 512], F32, tag="l65")
                nc.scalar.activation(out=l65[:], in_=pov[:],
                                     func=mybir.ActivationFunctionType.Copy)
                nc.vector.reciprocal(out=l65[64:65, :], in_=l65[64:65, :])
                ldr = dp.tile([1, 512], F32, tag="ldr")
                nc.sync.dma_start(out=ldr[:], in_=l65[64:65, :])
                lbc = sm2.tile([64, 512], F32, tag="lbc")
                nc.sync.dma_start(out=lbc[:], in_=ldr[:].to_broadcast((64, 512)))
                if qrow == 0:
                    nc.vector.tensor_mul(out=attnT[0:64, qsub, qs0:qs0 + 512],
                                         in0=l65[0:64, :], in1=lbc[:])
                else:
                    nrm = sm2.tile([64, 512], F32R, tag="nrm")
                    nc.vector.tensor_mul(out=nrm[:], in0=l65[0:64, :], in1=lbc[:])
                    nc.gpsimd.dma_start(out=attnT[64:128, qsub, qs0:qs0 + 512],
                                        in_=nrm[:])

            W = 4
            pend_av = []
            step = [0]

            def pump(now=False):
                while pend_av and (now or pend_av[0][0] <= step[0]):
                    _, pov, ks, last, pt, r, norm = pend_av.pop(0)
                    nc.tensor.matmul(pov[:, r:512], lhsT=vp[:, ks, :],
                                     rhs=pt[:, r:512],
                                     start=(ks == 0), stop=(ks == last))
                    if norm is not None:
                        norm_out(pov, *norm)

            for qt_i in range(NCH):
                for qsub in range(2):
                    qs0 = qt_i * 512
                    nsteps = 4 * qt_i + 4
                    povs = [po.tile([65, 512], F32, tag="o", name=f"pov{i}")
                            for i in range(2)]
                    for ks in range(nsteps):
                        r = max(0, ks * 128 - qs0)
                        n2 = 512 - r
                        for h2 in range(2):
                            qrow = h2 * 64
                            ps_a = pa.tile([128, 512], F32, tag="a")
                            nc.tensor.matmul(
                                ps_a[:, 0:n2],
                                lhsT=kTd[qrow:qrow + 64, ks * 128:(ks + 1) * 128],
                                rhs=qT[qrow:qrow + 64, qsub, qs0 + r:qs0 + 512],
                                start=True, stop=True)
                            pt = ptp.tile([128, 512], F32R, tag="ptk")
                            nc.scalar.activation(out=pt[:, r:512], in_=ps_a[:, 0:n2],
                                                 func=EXP, scale=0.125)
                            if ks >= 4 * qt_i:
                                nc.gpsimd.tensor_mul(out=pt[:, r:r + 128],
                                                     in0=pt[:, r:r + 128],
                                                     in1=m0[:])
                            norm = ((qrow, qsub, qs0)
                                    if ks == nsteps - 1 else None)
                            pend_av.append((step[0] + W, povs[h2], ks,
                                            nsteps - 1, pt, r, norm))
                        step[0] += 1
                        pump()
                if qt_i >= 1:
                    emit_outproj(b, attnT, qt_i - 1)
            pump(now=True)
            emit_outproj(b, attnT, NCH - 1)
        ctx.close()
    nc.compile()
    return nc


def _get_nc():
    global _NC
    if _NC is None:
        _NC = _build()
    return _NC


def kernel(**inputs) -> np.ndarray:
    global LAST_EXEC_NS
    hidden = np.asarray(inputs["hidden_states"], np.float32)
    cos = np.asarray(inputs["cos"], np.float32)
    sin = np.asarray(inputs["sin"], np.float32)
    W_q = np.asarray(inputs["W_q"], np.float32)
    W_k = np.asarray(inputs["W_k"], np.float32)
    W_v = np.asarray(inputs["W_v"], np.float32)
    b_v = np.asarray(inputs["b_v"], np.float32)
    W_o = np.asarray(inputs["W_o"], np.float32)
    b_o = np.asarray(inputs["b_o"], np.float32)
    # attention_mask is all-ones by construction (spec fill "ones"); the
    # causal mask is applied in-kernel.

    xt = np.ascontiguousarray(
        hidden.reshape(BS, DM).T.reshape(KSUB, 128, BS).transpose(1, 0, 2))
    cosT = cos.reshape(BS, HD).T
    sinT = sin.reshape(BS, HD).T
    cos2t = np.ascontiguousarray(np.concatenate([cosT, cosT], 0))
    sin2t = np.ascontiguousarray(np.concatenate([sinT, sinT], 0))

    RT = np.zeros((64, 64), np.float32)
    for dd in range(32):
        RT[dd + 32, dd] = -1.0
        RT[dd, dd + 32] = 1.0
    rt = np.zeros((128, 128), np.float32)
    rt[0:64, 0:64] = RT
    rt[64:128, 64:128] = RT
    idl = np.zeros((128, 64), np.float32)
    idl[64:128, :] = np.eye(64, dtype=np.float32)
    m0 = np.triu(np.ones((128, 128), np.float32))
    on16 = np.ones((128, SK), np.float32)

    in_maps = []
    for c in range(NCORES):
        wq_c = np.ascontiguousarray(
            W_q[:, c * 256:(c + 1) * 256].reshape(KSUB, 128, 256).transpose(1, 0, 2))
        wkv_c = np.ascontiguousarray(
            np.concatenate([W_k[:, c * 64:(c + 1) * 64],
                            W_v[:, c * 64:(c + 1) * 64]], 1)
            .reshape(KSUB, 128, 128).transpose(1, 0, 2))
        wo_c = np.ascontiguousarray(
            W_o[c * 256:(c + 1) * 256, :].reshape(2, 128, DM).transpose(1, 0, 2))
        bv_c = np.zeros((128, 1), np.float32)
        bv_c[64:128, 0] = b_v[c * 64:(c + 1) * 64]
        in_maps.append({
            "xt": xt, "wq": wq_c, "wkv": wkv_c, "wo": wo_c,
            "cos2t": cos2t, "sin2t": sin2t, "bv": bv_c, "rt": rt,
            "idl": idl, "m0": m0, "on16": on16,
        })

    nc = _get_nc()
    res = run_bass_kernel_spmd(nc, in_maps, core_ids=list(range(NCORES)))
    LAST_EXEC_NS = res.exec_time_ns

    total = np.zeros((BS, DM), np.float64)
    for c in range(NCORES):
        total += res.results[c]["out"].astype(np.float64)
    total += b_o.astype(np.float64)
    return total.reshape(B, S, DM).astype(np.float32)


# revision 35
# speedup vs baseline: 1.2784x; 1.2784x over previous
"""GQA attention layer (B=2, S=2048, DM=2048, H=32, KV=8, HD=64) with RoPE,
causal mask and output projection, tensor-parallel over heads across 8
Trainium2 NeuronCores.

Sharding: core c owns query heads [4c, 4c+4) and KV head c.  Each core
computes its 4 heads' attention plus its slice of the output projection
(W_o rows [256c, 256c+256)); the host sums the 8 partial outputs and adds
b_o.

Layout strategy: everything on-chip lives transposed (feature dim on
partitions) so that Q/K projections, RoPE rotation (a PE matmul against a
+-1 permutation matrix), scores A^T = K^T.T @ Q^T, P^T = exp(A^T/8), and
O'^T = V'.T @ P^T chain together without any transposes except V (done
with 16 small PE transposes per batch).  The softmax denominator rides as
a ones-column appended to V.  Matmuls run in float32r (~1.6e-4 rel err,
4x the fp32 rate).
"""

import os
import numpy as np

import concourse.bass as bass
import concourse.tile as tile
from concourse import bacc, mybir
from concourse.bass_utils import run_bass_kernel_spmd

F32 = mybir.dt.float32
F32R = mybir.dt.float32r
EXP = mybir.ActivationFunctionType.Exp
MULT = mybir.AluOpType.add  # placeholder (unused)

B, S, DM = 2, 2048, 2048
H, KV, HD = 32, 8, 64
NCORES = 8
HPC = H // NCORES            # query heads per core
BS = B * S
KSUB = DM // 128             # 16 contraction subtiles
NCH = S // 512               # 4 seq chunks of 512 per batch
SK = S // 128                # 16 key subtiles per batch

_NC = None
LAST_EXEC_NS = None


def _build():
    nc = bacc.Bacc("TRN2", target_bir_lowering=False, debug=False)
    d = {}
    d["xt"] = nc.declare_dram_parameter("xt", [128, KSUB, BS], F32R, isOutput=False)
    d["wq"] = nc.declare_dram_parameter("wq", [128, KSUB, 256], F32R, isOutput=False)
    d["wkv"] = nc.declare_dram_parameter("wkv", [128, KSUB, 128], F32R, isOutput=False)
    d["wo"] = nc.declare_dram_parameter("wo", [128, 2, DM], F32R, isOutput=False)
    d["cos2t"] = nc.declare_dram_parameter("cos2t", [128, BS], F32, isOutput=False)
    d["sin2t"] = nc.declare_dram_parameter("sin2t", [128, BS], F32, isOutput=False)
    d["bv"] = nc.declare_dram_parameter("bv", [128, 1], F32, isOutput=False)
    d["rt"] = nc.declare_dram_parameter("rt", [128, 128], F32R, isOutput=False)
    d["idl"] = nc.declare_dram_parameter("idl", [128, 64], F32R, isOutput=False)
    d["m0"] = nc.declare_dram_parameter("m0", [128, 128], F32R, isOutput=False)
    d["on16"] = nc.declare_dram_parameter("on16", [128, SK], F32R, isOutput=False)
    out_d = nc.declare_dram_parameter("out", [BS, DM], F32, isOutput=True)

    import contextlib
    with tile.TileContext(nc) as tc:
        ctx = contextlib.ExitStack()
        cst = ctx.enter_context(tc.tile_pool(name="cst", bufs=1))
        act = ctx.enter_context(tc.tile_pool(name="act", bufs=1))
        qTp = ctx.enter_context(tc.tile_pool(name="qTp", bufs=1))
        big = ctx.enter_context(tc.tile_pool(name="big", bufs=2))
        ptp = ctx.enter_context(tc.tile_pool(name="ptp", bufs=5))
        sm1 = ctx.enter_context(tc.tile_pool(name="sm1", bufs=1))
        sm2 = ctx.enter_context(tc.tile_pool(name="sm2", bufs=2))
        ost_p = ctx.enter_context(tc.tile_pool(name="ostp", bufs=2))
        dp = ctx.enter_context(tc.tile_pool(name="dp", bufs=2, space="DRAM"))
        pmm = ctx.enter_context(tc.tile_pool(name="pmm", bufs=2, space="PSUM"))
        pa = ctx.enter_context(tc.tile_pool(name="pa", bufs=3, space="PSUM"))
        po = ctx.enter_context(tc.tile_pool(name="po", bufs=3, space="PSUM"))

        wq = cst.tile([128, KSUB, 256], F32R)
        wkv = cst.tile([128, KSUB, 128], F32R)
        wo = cst.tile([128, 2, DM], F32R)
        bv = cst.tile([128, 1], F32)
        rt = cst.tile([128, 128], F32R)
        idl = cst.tile([128, 64], F32R)
        m0 = cst.tile([128, 128], F32R)
        nc.sync.dma_start(out=wq[:, 0:8], in_=d["wq"][:, 0:8])
        nc.scalar.dma_start(out=wq[:, 8:16], in_=d["wq"][:, 8:16])
        nc.gpsimd.dma_start(out=wkv[:], in_=d["wkv"][:])
        for t, key in ((wo, "wo"), (bv, "bv"), (rt, "rt"), (idl, "idl"),
                       (m0, "m0")):
            nc.gpsimd.dma_start(out=t[:], in_=d[key][:])

        CW = 512  # QKV streaming chunk width

        def rope(dst, psrc, rows, cosc, sinc, rt_sl):
            """dst[rows, cols] = psrc*cos + rot(psrc)*sin, rot via PE."""
            sb = sm2.tile([128, CW], F32R, tag="qsb")
            nc.vector.tensor_copy(out=sb[0:rows, :], in_=psrc)
            prot = pa.tile([128, 512], F32, tag="a")
            nc.tensor.matmul(prot[0:rows, 0:CW], lhsT=rt_sl, rhs=sb[0:rows, :],
                             start=True, stop=True)
            nc.vector.tensor_mul(out=dst, in0=sb[0:rows, :], in1=cosc[0:rows, :])
            rots = sm2.tile([128, CW], F32, tag="rots")
            nc.vector.tensor_mul(out=rots[0:rows, :], in0=prot[0:rows, 0:CW],
                                 in1=sinc[0:rows, :])
            nc.vector.tensor_add(out=dst, in0=dst, in1=rots[0:rows, :])

        def emit_outproj(b, attnT, qt_i):
            for ss in range(qt_i * 4, qt_i * 4 + 4):
                for oc in range(DM // 512):
                    ps = pmm.tile([128, 512], F32, tag="mm")
                    for k2 in range(2):
                        nc.tensor.matmul(ps[:],
                                         lhsT=attnT[:, k2, ss * 128:(ss + 1) * 128],
                                         rhs=wo[:, k2, oc * 512:(oc + 1) * 512],
                                         start=(k2 == 0), stop=(k2 == 1))
                    ost = ost_p.tile([128, 512], F32, tag="ost")
                    nc.vector.tensor_copy(out=ost[:], in_=ps[:])
                    nc.scalar.dma_start(
                        out=out_d[b * S + ss * 128:b * S + (ss + 1) * 128,
                                  oc * 512:(oc + 1) * 512],
                        in_=ost[:])

        deferred = []
        for b in range(B):
            qT = qTp.tile([128, 2, S], F32R, tag="qT")
            kTd = act.tile([128, S], F32R, tag="kTd")
            vp = act.tile([128, SK, 65], F32R, tag="vp")
            nc.gpsimd.dma_start(out=vp[:, :, 64], in_=d["on16"][:])
            attnT = act.tile([128, 2, S], F32R, tag="attnT")

            # ---- QKV projections + RoPE + V transpose ----
            for sc in range(S // CW):
                col0 = b * S + sc * CW
                cs = slice(sc * CW, sc * CW + CW)
                xt = big.tile([128, KSUB, CW], F32R, tag="big")
                nc.sync.dma_start(out=xt[:, 0:8], in_=d["xt"][:, 0:8, col0:col0 + CW])
                nc.scalar.dma_start(out=xt[:, 8:16],
                                    in_=d["xt"][:, 8:16, col0:col0 + CW])
                cosc = sm2.tile([128, CW], F32, tag="cos")
                sinc = sm2.tile([128, CW], F32, tag="sin")
                nc.gpsimd.dma_start(out=cosc[:], in_=d["cos2t"][:, col0:col0 + CW])
                nc.gpsimd.dma_start(out=sinc[:], in_=d["sin2t"][:, col0:col0 + CW])
                ps = pmm.tile([128, CW], F32, tag="mm")
                for ks in range(KSUB):
                    nc.tensor.matmul(ps[:], lhsT=wkv[:, ks, :], rhs=xt[:, ks, :],
                                     start=(ks == 0), stop=(ks == KSUB - 1))
                rope(kTd[0:64, cs], ps[0:64, :], 64, cosc, sinc, rt[0:64, 0:64])
                nc.sync.dma_start(out=kTd[64:128, cs], in_=kTd[0:64, cs])
                vtt = sm2.tile([128, CW], F32R, tag="vtt")
                nc.vector.tensor_add(out=vtt[64:128, :], in0=ps[64:128, :],
                                     in1=bv[64:128, 0:1].to_broadcast((64, CW)))
                for t4 in range(CW // 128):
                    pvt = pa.tile([128, 64], F32R, tag="a")
                    nc.tensor.matmul(pvt[:], lhsT=vtt[64:128, t4 * 128:(t4 + 1) * 128],
                                     rhs=idl[64:128, :], is_transpose=True,
                                     start=True, stop=True)
                    nc.vector.tensor_copy(
                        out=vp[:, (sc * CW) // 128 + t4, 0:64], in_=pvt[:])
                for m in range(2):
                    ps = pmm.tile([128, CW], F32, tag="mm")
                    for ks in range(KSUB):
                        nc.tensor.matmul(ps[:], lhsT=wq[:, ks, m * 128:(m + 1) * 128],
                                         rhs=xt[:, ks, :],
                                         start=(ks == 0), stop=(ks == KSUB - 1))
                    rope(qT[:, m, cs], ps[:], 128, cosc, sinc, rt[:])

            # ---- attention: 2 head chains per qt, AV trails by one step;
            # a scalar-engine copy frees each accumulator bank fast ----
            def norm_out(pov, qrow, qsub, qs0):
                l65 = sm2.tile([65, 512], F32, tag="l65")
                nc.scalar.activation(out=l65[:], in_=pov[:],
                                     func=mybir.ActivationFunctionType.Copy)
                nc.vector.reciprocal(out=l65[64:65, :], in_=l65[64:65, :])
                ldr = dp.tile([1, 512], F32, tag="ldr")
                nc.sync.dma_start(out=ldr[:], in_=l65[64:65, :])
                lbc = sm2.tile([64, 512], F32, tag="lbc")
                nc.sync.dma_start(out=lbc[:], in_=ldr[:].to_broadcast((64, 512)))
                if qrow == 0:
                    nc.vector.tensor_mul(out=attnT[0:64, qsub, qs0:qs0 + 512],
                                         in0=l65[0:64, :], in1=lbc[:])
                else:
                    nrm = sm2.tile([64, 512], F32R, tag="nrm")
                    nc.vector.tensor_mul(out=nrm[:], in0=l65[0:64, :], in1=lbc[:])
                    nc.gpsimd.dma_start(out=attnT[64:128, qsub, qs0:qs0 + 512],
                                        in_=nrm[:])

            for qt_i in range(NCH):
                for qsub in range(2):
                    qs0 = qt_i * 512
                    nsteps = 4 * qt_i + 4
                    povs = [po.tile([65, 512], F32, tag="o", name=f"pov{i}")
                            for i in range(2)]
                    pend = {}
                    def emit_av(ks, h2):
                        pt, r = pend.pop((ks, h2))
                        nc.tensor.matmul(povs[h2][:, r:512], lhsT=vp[:, ks, :],
                                         rhs=pt[:, r:512],
                                         start=(ks == 0), stop=(ks == nsteps - 1))
                    for ks in range(nsteps):
                        r = max(0, ks * 128 - qs0)
                        n2 = 512 - r
                        for h2 in range(2):
                            qrow = h2 * 64
                            ps_a = pa.tile([128, 512], F32, tag="a")
                            nc.tensor.matmul(
                                ps_a[:, 0:n2],
                                lhsT=kTd[qrow:qrow + 64, ks * 128:(ks + 1) * 128],
                                rhs=qT[qrow:qrow + 64, qsub, qs0 + r:qs0 + 512],
                                start=True, stop=True)
                            pt = ptp.tile([128, 512], F32R, tag="ptk")
                            nc.scalar.activation(out=pt[:, r:512], in_=ps_a[:, 0:n2],
                                                 func=EXP, scale=0.125)
                            if ks >= 4 * qt_i:
                                nc.gpsimd.tensor_mul(out=pt[:, r:r + 128],
                                                     in0=pt[:, r:r + 128],
                                                     in1=m0[:])
                            pend[(ks, h2)] = (pt, r)
                        if ks >= 1:
                            for h2 in range(2):
                                emit_av(ks - 1, h2)
                    for h2 in range(2):
                        emit_av(nsteps - 1, h2)
                    for h2 in range(2):
                        norm_out(povs[h2], h2 * 64, qsub, qs0)
                emit_outproj(b, attnT, qt_i)
        ctx.close()
    nc.compile()
    return nc


def _get_nc():
    global _NC
    if _NC is None:
        _NC = _build()
    return _NC


def kernel(**inputs) -> np.ndarray:
    global LAST_EXEC_NS
    hidden = np.asarray(inputs["hidden_states"], np.float32)
    cos = np.asarray(inputs["cos"], np.float32)
    sin = np.asarray(inputs["sin"], np.float32)
    W_q = np.asarray(inputs["W_q"], np.float32)
    W_k = np.asarray(inputs["W_k"], np.float32)
    W_v = np.asarray(inputs["W_v"], np.float32)
    b_v = np.asarray(inputs["b_v"], np.float32)
    W_o = np.asarray(inputs["W_o"], np.float32)
    b_o = np.asarray(inputs["b_o"], np.float32)
    # attention_mask is all-ones by construction (spec fill "ones"); the
    # causal mask is applied in-kernel.

    xt = np.ascontiguousarray(
        hidden.reshape(BS, DM).T.reshape(KSUB, 128, BS).transpose(1, 0, 2))
    cosT = cos.reshape(BS, HD).T
    sinT = sin.reshape(BS, HD).T
    cos2t = np.ascontiguousarray(np.concatenate([cosT, cosT], 0))
    sin2t = np.ascontiguousarray(np.concatenate([sinT, sinT], 0))

    RT = np.zeros((64, 64), np.float32)
    for dd in range(32):
        RT[dd + 32, dd] = -1.0
        RT[dd, dd + 32] = 1.0
    rt = np.zeros((128, 128), np.float32)
    rt[0:64, 0:64] = RT
    rt[64:128, 64:128] = RT
    idl = np.zeros((128, 64), np.float32)
    idl[64:128, :] = np.eye(64, dtype=np.float32)
    m0 = np.triu(np.ones((128, 128), np.float32))
    on16 = np.ones((128, SK), np.float32)

    in_maps = []
    for c in range(NCORES):
        wq_c = np.ascontiguousarray(
            W_q[:, c * 256:(c + 1) * 256].reshape(KSUB, 128, 256).transpose(1, 0, 2))
        wkv_c = np.ascontiguousarray(
            np.concatenate([W_k[:, c * 64:(c + 1) * 64],
                            W_v[:, c * 64:(c + 1) * 64]], 1)
            .reshape(KSUB, 128, 128).transpose(1, 0, 2))
        wo_c = np.ascontiguousarray(
            W_o[c * 256:(c + 1) * 256, :].reshape(2, 128, DM).transpose(1, 0, 2))
        bv_c = np.zeros((128, 1), np.float32)
        bv_c[64:128, 0] = b_v[c * 64:(c + 1) * 64]
        in_maps.append({
            "xt": xt, "wq": wq_c, "wkv": wkv_c, "wo": wo_c,
            "cos2t": cos2t, "sin2t": sin2t, "bv": bv_c, "rt": rt,
            "idl": idl, "m0": m0, "on16": on16,
        })

    nc = _get_nc()
    res = run_bass_kernel_spmd(nc, in_maps, core_ids=list(range(NCORES)))
    LAST_EXEC_NS = res.exec_time_ns

    total = np.zeros((BS, DM), np.float64)
    for c in range(NCORES):
        total += res.results[c]["out"].astype(np.float64)
    total += b_o.astype(np.float64)
    return total.reshape(B, S, DM).astype(np.float32)
